# revision 1
# baseline (speedup 1.0000x reference)
"""Trainium2 Bass kernel for nn_Decoder (Bahdanau-attention LSTM decoder).

B=256,T=128,ENC=DEC=256,OUT=3. Data-parallel over batch: 8 cores x 32 batch.

Per-core design (feature-major attention pipeline, batch-major LSTM):
  - z2 = W2 @ X^T precomputed once into SBUF, bf16, free order (t,b) "t-major"
  - per step: z1 (PE, f-major) -> broadcast-add over t (DVE 2x bf16) ->
    tanh (ACT) -> scores via 32 accumulating MMs with diagonal-masked w3
    lhsT -> psum [32,128] b-major -> exp+rowsum (ACT fused accum) ->
    E^T (PE transpose) -> diag-write E into arena -> ctx via 32 accumulating
    MMs -> psum [32,256] -> scale by 1/D (DVE) -> transposes -> gates MM
    (fp32r, weights streamed) -> LSTM elementwise -> state transposes.
  - total_hidden stored f-major in SBUF; head (fc2@fc1 composed on host) is
    one fp32r matmul sweep at the end.
"""

import sys
import numpy as np

sys.path.insert(0, "/opt/trn_rl_repo")

import ml_dtypes

BF16 = ml_dtypes.bfloat16

NCORES = 8
BL = 32          # batch per core
T = 128          # encoder positions == decoder steps
ENC = 256
DEC = 256
OUT = 3
BT = BL * T      # 4096
S = 128          # decoder steps

_BUILT = None


def _build_nc():
    from contextlib import ExitStack
    from concourse import bacc, mybir, tile

    dt = mybir.dt
    F32, B16, F32R = dt.float32, dt.bfloat16, dt.float32r
    AF = mybir.ActivationFunctionType

    nc = bacc.Bacc("TRN2", target_bir_lowering=False, debug=False,
                   enable_asserts=False, num_devices=NCORES)

    # ---- DRAM I/O ----
    di = lambda n, sh, d: nc.dram_tensor(n, sh, d, kind="ExternalInput").ap()
    xt = di("xt", [ENC, BT], B16)        # X^T, cols t-major: [e, t*32+b]
    x = di("x", [BT, ENC], B16)          # X, rows b-major: [b*128+t, e]
    y = di("y", [OUT, S * BL], F32R)      # [j, s*32+b]
    w2t = di("w2t", [ENC, ENC], B16)     # attn2_w.T [e, f]
    w1t = di("w1t", [2 * DEC, ENC], B16)  # attn1_w.T [k_hc, f]
    w3d = di("w3d", [128, 2048], B16)    # diag-masked w3 [f_row, fc*1024+b*32+j]
    bc = di("bc", [ENC, 1], F32)         # attn1_b + attn2_b
    wcy = di("wcy", [OUT, 4 * DEC], F32R)     # W_comb.T rows 0:3
    wcc = di("wcc", [ENC, 4 * DEC], F32R)     # W_comb.T rows 3:259
    whht = di("whht", [DEC, 4 * DEC], F32R)   # w_hh.T
    gb = di("gb", [1, 4 * DEC], F32R)
    fct = di("fct", [DEC + ENC, OUT], F32R)   # (fc2@fc1).T
    fcb = di("fcb", [1, OUT], F32R)
    onesr = di("onesr", [1, 512], F32R)
    i32 = di("i32", [32, 32], F32)           # identity for transposes
    o = nc.dram_tensor("o", [OUT, S * BL], dt.float32, kind="ExternalOutput").ap()

    with tile.TileContext(nc) as tc, ExitStack() as ctx:
        # ---------------- persistent SBUF ----------------
        P = ctx.enter_context(tc.tile_pool(name="persist", bufs=1))
        Z2 = [P.tile([128, BT], B16, tag=f"z2{i}", name=f"Z2_{i}") for i in range(2)]
        XS = P.tile([128, BL * ENC], B16, tag="xs")          # [t, b*256+e]
        YS = P.tile([OUT, S * BL], F32R, tag="ys")
        W1TS = P.tile([128, 4 * ENC], B16, tag="w1ts")       # [kc*256+f]
        W3DS = P.tile([128, 2048], B16, tag="w3ds")
        BCS = P.tile([128, 2], F32, tag="bcs")
        WCYS = P.tile([OUT, 4 * DEC], F32R, tag="wcys")
        WCCS = P.tile([128, 2 * 4 * DEC], F32R, tag="wccs")
        WHHTS = P.tile([128, 2 * 4 * DEC], F32R, tag="whhts")
        GBS = P.tile([1, 4 * DEC], F32R, tag="gbs")
        FCTS = P.tile([128, 4 * OUT], F32R, tag="fcts")
        FCBS = P.tile([1, OUT], F32R, tag="fcbs")
        ONES = P.tile([1, 512], F32R, tag="ones")
        I32F = P.tile([32, 32], F32, tag="i32f")
        TH = [P.tile([128, S * BL], F32R, tag=f"th{i}", name=f"TH_{i}") for i in range(4)]
        DIAG = P.tile([128, 32 * 32], B16, tag="diag")       # ctx lhsT arena
        HCT0 = P.tile([128, 128], B16, tag="hct0")           # zero h,c^T step0
        Z128 = P.tile([128, 64], F32R, tag="z128")            # zero h^T fp32 step0
        C0 = P.tile([BL, DEC], F32, tag="c0")

        # load weights / inputs
        for b in range(BL):
            nc.sync.dma_start(XS[:, b * ENC:(b + 1) * ENC], x[b * T:(b + 1) * T, :])
        nc.sync.dma_start(YS[:], y[:])
        for kc in range(4):
            nc.sync.dma_start(W1TS[:, kc * ENC:(kc + 1) * ENC],
                              w1t[kc * 128:(kc + 1) * 128, :])
        nc.sync.dma_start(W3DS[:], w3d[:])
        for c in range(2):
            nc.sync.dma_start(BCS[:, c:c + 1], bc[c * 128:(c + 1) * 128, :])
        nc.sync.dma_start(WCYS[:], wcy[:])
        for j in range(2):
            nc.sync.dma_start(WCCS[:, j * 1024:(j + 1) * 1024],
                              wcc[j * 128:(j + 1) * 128, :])
            nc.sync.dma_start(WHHTS[:, j * 1024:(j + 1) * 1024],
                              whht[j * 128:(j + 1) * 128, :])
        nc.sync.dma_start(GBS[:], gb[:])
        for kc in range(4):
            nc.sync.dma_start(FCTS[:, kc * OUT:(kc + 1) * OUT],
                              fct[kc * 128:(kc + 1) * 128, :])
        nc.sync.dma_start(FCBS[:], fcb[:])
        nc.sync.dma_start(ONES[:], onesr[:])
        nc.sync.dma_start(I32F[:], i32[:])

        nc.vector.memset(DIAG[:], 0.0)
        nc.vector.memset(HCT0[:], 0.0)
        nc.vector.memset(Z128[:].bitcast(F32), 0.0)
        nc.vector.memset(C0[:], 0.0)

        # ---------------- z2 precompute ----------------
        with tc.tile_pool(name="xts", bufs=1) as xtp, \
             tc.tile_pool(name="z2ps", bufs=2, space="PSUM") as z2ps, \
             tc.tile_pool(name="w2p", bufs=1) as w2p:
            W2TS = w2p.tile([128, 2 * ENC], B16)
            for ec in range(2):
                nc.sync.dma_start(W2TS[:, ec * ENC:(ec + 1) * ENC],
                                  w2t[ec * 128:(ec + 1) * 128, :])
            XTS = [xtp.tile([128, BT], B16, tag=f"xt{e}", name=f"XTS_{e}") for e in range(2)]
            for ec in range(2):
                nc.sync.dma_start(XTS[ec][:], xt[ec * 128:(ec + 1) * 128, :])
            for fc in range(2):
                for nq in range(8):
                    zp = z2ps.tile([128, 512], F32, tag="zp")
                    for ec in range(2):
                        nc.tensor.matmul(
                            zp[:], W2TS[:, ec * ENC + fc * 128: ec * ENC + fc * 128 + 128],
                            XTS[ec][:, nq * 512:(nq + 1) * 512],
                            start=(ec == 0), stop=(ec == 1))
                    nc.vector.tensor_copy(Z2[fc][:, nq * 512:(nq + 1) * 512], zp[:])

        # ---------------- step pools ----------------
        loop_ctx = ExitStack()
        tin_p = loop_ctx.enter_context(tc.tile_pool(name="tin", bufs=1))
        tout_p = loop_ctx.enter_context(tc.tile_pool(name="tout", bufs=2))
        sb_p = loop_ctx.enter_context(tc.tile_pool(name="small", bufs=2))
        st_p = loop_ctx.enter_context(tc.tile_pool(name="state", bufs=2))
        sc_ps = loop_ctx.enter_context(tc.tile_pool(name="scps", bufs=1, space="PSUM"))
        cx_ps = loop_ctx.enter_context(tc.tile_pool(name="cxps", bufs=1, space="PSUM"))
        g_ps = loop_ctx.enter_context(tc.tile_pool(name="gps", bufs=1, space="PSUM"))
        z1_ps = loop_ctx.enter_context(tc.tile_pool(name="z1ps", bufs=1, space="PSUM"))
        tp_ps = loop_ctx.enter_context(tc.tile_pool(name="tpps", bufs=2, space="PSUM"))

        hct_prev = HCT0          # [128,128] bf16: h^T(2) ++ c^T(2) blocks of 32 cols
        hT_prev = None           # fp32 h^T for gates: TH slots or Z128
        c_prev = C0

        r32 = lambda ap: ap.bitcast(F32R)

        for s in range(S):
            last = (s == S - 1)
            # ---- z1 = W1 @ hc + (b1+b2), f-major [f, b] ----
            z1p = z1_ps.tile([128, 64], F32, tag="z1")
            for fc in range(2):
                for kc in range(4):
                    nc.tensor.matmul(
                        z1p[:, fc * 32:(fc + 1) * 32],
                        W1TS[:, kc * ENC + fc * 128: kc * ENC + fc * 128 + 128],
                        hct_prev[:, kc * 32:(kc + 1) * 32],
                        start=(kc == 0), stop=(kc == 3))
            z1s = sb_p.tile([128, 64], B16, tag="z1s")
            for fc in range(2):
                nc.scalar.activation(z1s[:, fc * 32:(fc + 1) * 32],
                                     z1p[:, fc * 32:(fc + 1) * 32],
                                     AF.Identity, bias=BCS[:, fc:fc + 1])

            # ---- tanh(z1 + z2): DVE bcast add (t-major -> 2x mode) + ACT ----
            touts = []
            for fc in range(2):
                tin = tin_p.tile([128, BT], B16, tag="tin")
                tin3 = tin[:].rearrange("p (t b) -> p t b", b=32)
                z23 = Z2[fc][:].rearrange("p (t b) -> p t b", b=32)
                z1b = z1s[:, None, fc * 32:(fc + 1) * 32].broadcast_to([128, T, 32])
                nc.vector.tensor_add(tin3, z23, z1b)
                tout = tout_p.tile([128, BT], B16, tag="tout")
                nc.scalar.activation(tout[:], tin[:], AF.Tanh)
                touts.append(tout)

            # ---- scores: 64 accumulating diag-lhsT MMs -> psum [32,128] ----
            scp = sc_ps.tile([32, 128], F32, tag="sc")
            for fc in range(2):
                t3 = touts[fc][:].rearrange("p (t b) -> p t b", b=32)
                for b in range(BL):
                    nc.tensor.matmul(
                        scp[:], W3DS[:, fc * 1024 + b * 32: fc * 1024 + b * 32 + 32],
                        t3[:, :, b],
                        start=(fc == 0 and b == 0), stop=(fc == 1 and b == BL - 1))

            # ---- softmax pieces ----
            E = sb_p.tile([32, 128], F32, tag="E")
            D = sb_p.tile([32, 1], F32, tag="D")
            nc.scalar.activation(E[:], scp[:], AF.Exp, accum_out=D[:])
            Dinv = sb_p.tile([32, 1], F32, tag="Dinv")
            nc.vector.reciprocal(Dinv[:], D[:])

            # E^T via PE transpose, then write diagonal of ctx-lhsT arena:
            # dst cols b*32+b, i.e. flat stride 33
            arena = tp_ps.tile([128, 256], F32, tag="arena")
            nc.tensor.transpose(arena[:, 0:32], E[:], I32F[:])
            nc.vector.tensor_copy(DIAG[:, 0:32 * 32:33], arena[:, 0:32])

            # ---- context: 32 accumulating MMs -> psum [32,256] b-major ----
            cxp = cx_ps.tile([32, ENC], F32, tag="cx")
            for b in range(BL):
                nc.tensor.matmul(
                    cxp[:], DIAG[:, b * 32:(b + 1) * 32],
                    XS[:, b * ENC:(b + 1) * ENC],
                    start=(b == 0), stop=(b == BL - 1))
            ctxb = sb_p.tile([BL, ENC], F32, tag="ctxb")
            nc.vector.tensor_scalar_mul(ctxb[:], cxp[:], Dinv[:])

            # ctx^T into TH (f-major), also gates lhsT
            for j in range(2):
                nc.tensor.transpose(arena[:, 32 + j * 32: 64 + j * 32],
                                    ctxb[:, j * 128:(j + 1) * 128], I32F[:])
                nc.vector.tensor_copy(TH[2 + j][:, s * 32:(s + 1) * 32],
                                      arena[:, 32 + j * 32: 64 + j * 32])

            if last:
                # h2_127 == h_127: copy previous th h-slots
                for j in range(2):
                    nc.vector.tensor_copy(TH[j][:, s * 32:(s + 1) * 32],
                                          TH[j][:, (s - 1) * 32: s * 32])
                break

            # ---- gates: psum [32, 1024], fp32r streams ----
            gp = g_ps.tile([BL, 4 * DEC], F32, tag="g")
            for nh in range(2):
                c0, c1 = nh * 512, nh * 512 + 512
                nc.tensor.matmul(gp[:, c0:c1], ONES[:, 0:32],
                                 GBS[:, c0:c1], start=True, stop=False)
                nc.tensor.matmul(gp[:, c0:c1],
                                 YS[:, s * 32:(s + 1) * 32],
                                 WCYS[:, c0:c1], start=False, stop=False)
                for j in range(2):
                    nc.tensor.matmul(gp[:, c0:c1],
                                     TH[2 + j][:, s * 32:(s + 1) * 32],
                                     WCCS[:, j * 1024 + c0: j * 1024 + c1],
                                     start=False, stop=False)
                for j in range(2):
                    hTj = (Z128[:, j * 32:(j + 1) * 32] if s == 0
                           else TH[j][:, (s - 1) * 32: s * 32])
                    nc.tensor.matmul(gp[:, c0:c1], hTj,
                                     WHHTS[:, j * 1024 + c0: j * 1024 + c1],
                                     start=False, stop=(j == 1))

            # ---- LSTM elementwise (b-major [32, .]) ----
            sif = st_p.tile([BL, 512], F32, tag="sif")
            nc.scalar.activation(sif[:], gp[:, 0:512], AF.Sigmoid)
            tg = st_p.tile([BL, DEC], F32, tag="tg")
            nc.scalar.activation(tg[:], gp[:, 512:768], AF.Tanh)
            so = st_p.tile([BL, DEC], F32, tag="so")
            nc.scalar.activation(so[:], gp[:, 768:1024], AF.Sigmoid)
            t1 = st_p.tile([BL, DEC], F32, tag="t1")
            nc.vector.tensor_mul(t1[:], sif[:, 256:512], c_prev[:])
            t2 = st_p.tile([BL, DEC], F32, tag="t2")
            nc.vector.tensor_mul(t2[:], sif[:, 0:256], tg[:])
            cn = st_p.tile([BL, DEC], F32, tag="cn")
            nc.vector.tensor_add(cn[:], t1[:], t2[:])
            tc_ = st_p.tile([BL, DEC], F32, tag="tc")
            nc.scalar.activation(tc_[:], cn[:], AF.Tanh)
            hn = st_p.tile([BL, DEC], F32, tag="hn")
            nc.vector.tensor_mul(hn[:], so[:], tc_[:])

            # ---- state transposes -> TH h-slots (fp32) + HCT bf16 ----
            hct = sb_p.tile([128, 128], B16, tag="hct")
            for j in range(2):
                nc.tensor.transpose(arena[:, 96 + j * 32: 128 + j * 32],
                                    hn[:, j * 128:(j + 1) * 128], I32F[:])
                nc.vector.tensor_copy(TH[j][:, s * 32:(s + 1) * 32],
                                      arena[:, 96 + j * 32: 128 + j * 32])
                nc.vector.tensor_copy(hct[:, j * 32:(j + 1) * 32],
                                      arena[:, 96 + j * 32: 128 + j * 32])
            for j in range(2):
                nc.tensor.transpose(arena[:, 160 + j * 32: 192 + j * 32],
                                    cn[:, j * 128:(j + 1) * 128], I32F[:])
                nc.vector.tensor_copy(hct[:, 64 + j * 32: 96 + j * 32],
                                      arena[:, 160 + j * 32: 192 + j * 32])

            hct_prev = hct
            c_prev = cn

        loop_ctx.close()

        # ---------------- output head ----------------
        with tc.tile_pool(name="ops", bufs=2, space="PSUM") as ops, \
             tc.tile_pool(name="ost", bufs=2) as ost:
            for nq in range(8):
                op = ops.tile([OUT, 512], F32, tag="op")
                for kc in range(4):
                    nc.tensor.matmul(op[:], FCTS[:, kc * OUT:(kc + 1) * OUT],
                                     TH[kc][:, nq * 512:(nq + 1) * 512],
                                     start=(kc == 0), stop=False)
                nc.tensor.matmul(op[:], FCBS[:], ONES[:],
                                 start=False, stop=True)
                ot = ost.tile([OUT, 512], F32, tag="ot")
                nc.vector.tensor_copy(ot[:], op[:])
                nc.sync.dma_start(o[:, nq * 512:(nq + 1) * 512], ot[:])

    nc.compile()
    return nc


def _host_prep(inputs):
    f32 = np.float32
    ie = np.asarray(inputs["input_encoded"], f32)      # [256,128,256]
    ys = np.asarray(inputs["y_seq"], f32)              # [256,128,3]
    a1w = np.asarray(inputs["attn1_w"], f32)           # [256,512]
    a1b = np.asarray(inputs["attn1_b"], f32)
    a2w = np.asarray(inputs["attn2_w"], f32)
    a2b = np.asarray(inputs["attn2_b"], f32)
    a3w = np.asarray(inputs["attn3_w"], f32)           # [1,256]
    tw = np.asarray(inputs["tilde_w"], f32)            # [512,259]
    tb = np.asarray(inputs["tilde_b"], f32)
    wih = np.asarray(inputs["w_ih"], f32)              # [1024,512]
    whh = np.asarray(inputs["w_hh"], f32)              # [1024,256]
    bih = np.asarray(inputs["b_ih"], f32)
    bhh = np.asarray(inputs["b_hh"], f32)
    f1w = np.asarray(inputs["fc1_w"], f32)             # [256,512]
    f1b = np.asarray(inputs["fc1_b"], f32)
    f2w = np.asarray(inputs["fc2_w"], f32)             # [3,256]
    f2b = np.asarray(inputs["fc2_b"], f32)

    wcomb = wih @ tw                                    # [1024,259]
    wcombT = np.ascontiguousarray(wcomb.T)              # [259,1024]
    gbias = wih @ tb + bih + bhh                        # [1024]
    fc = f2w @ f1w                                      # [3,512]
    fcbias = f2w @ f1b + f2b                            # [3]

    w3diag = np.zeros((128, 2, 32, 32), f32)
    for fc_ in range(2):
        w3diag[:, fc_, np.arange(32), np.arange(32)] = \
            a3w[0, fc_ * 128:(fc_ + 1) * 128][:, None]
    w3diag = w3diag.reshape(128, 2048)

    shared = {
        "w2t": np.ascontiguousarray(a2w.T).astype(BF16),
        "w1t": np.ascontiguousarray(a1w.T).astype(BF16),
        "w3d": w3diag.astype(BF16),
        "bc": (a1b + a2b)[:, None].astype(f32),
        "wcy": np.ascontiguousarray(wcombT[0:3]).astype(f32),
        "wcc": np.ascontiguousarray(wcombT[3:259]).astype(f32),
        "whht": np.ascontiguousarray(whh.T).astype(f32),
        "gb": gbias[None, :].astype(f32),
        "fct": np.ascontiguousarray(fc.T).astype(f32),
        "fcb": fcbias[None, :].astype(f32),
        "onesr": np.ones((1, 512), f32),
        "i32": np.eye(32, dtype=f32),
    }
    in_maps = []
    for i in range(NCORES):
        b0 = i * BL
        xe = ie[b0:b0 + BL]                            # [32,128,256]
        m = dict(shared)
        m["xt"] = np.ascontiguousarray(
            xe.transpose(2, 1, 0).reshape(ENC, BT)).astype(BF16)
        m["x"] = xe.reshape(BT, ENC).astype(BF16)
        m["y"] = np.ascontiguousarray(
            ys[b0:b0 + BL].transpose(2, 1, 0).reshape(OUT, S * BL)).astype(f32)
        in_maps.append(m)
    return in_maps


def kernel(**inputs):
    global _BUILT
    from concourse import bass_utils
    if _BUILT is None:
        _BUILT = _build_nc()
    nc = _BUILT
    import os
    in_maps = _host_prep(inputs)
    trace = bool(int(os.environ.get("KERNEL_TRACE", "0")))
    res = bass_utils.run_bass_kernel_spmd(nc, in_maps, core_ids=list(range(NCORES)),
                                          trace=trace)
    if trace:
        print(f"HW exec time: {res.exec_time_ns} ns  (mean {res.mean_exec_time_ns})")
        globals()['_LAST_RESULTS'] = res
    outs = []
    for i in range(NCORES):
        oc = res.results[i]["o"]                       # [3, 4096] (j, s*32+b)
        outs.append(oc.reshape(OUT, S, BL).transpose(2, 1, 0))
    return np.concatenate(outs, axis=0).astype(np.float32)


if __name__ == "__main__":
    rng = np.random.default_rng(0)
    pass



# revision 13
# speedup vs baseline: 2.1706x; 2.1706x over previous
"""Trainium2 Bass kernel for nn_Decoder (Bahdanau-attention LSTM decoder).

B=256,T=128,ENC=DEC=256,OUT=3. Data-parallel over batch: 8 cores x 32 batch.

v2 design (per core):
  - z2 = W2 @ X^T + (b1+b2) precomputed into SBUF bf16, free order (t,b).
  - per step: z1 (PE) -> bcast-add over t (DVE 2x bf16) -> tanh (ACT, 2
    t-chunks per f-half for early score start) -> scores via diag-masked w3
    lhsT matmuls 4-way column-packed with tile_position -> psum [128,128]
    (4 col-groups x 8 batches) -> exp+rowsum (ACT) -> E^T (PE transpose) ->
    diag-write (strided DVE copy) -> ctx 4-way column-packed -> psum
    [128,256] -> Dinv-scaled copy (ACT) -> ctx^T transposes -> gates
    computed TRANSPOSED ([g,b] f-major): y/bias/h parts early (during tanh),
    ctx part after -> LSTM elementwise in f-major (no state transposes).
  - total_hidden bf16 f-major in TH; head = one bf16 matmul sweep at end.
"""

import sys
import numpy as np

sys.path.insert(0, "/opt/trn_rl_repo")

import ml_dtypes

BF16 = ml_dtypes.bfloat16

NCORES = 8
BL = 32          # batch per core
T = 128          # encoder positions == decoder steps
ENC = 256
DEC = 256
OUT = 3
BT = BL * T      # 4096
S = 128          # decoder steps

_BUILT = None


def _build_nc():
    from contextlib import ExitStack
    from concourse import bacc, mybir, tile

    dt = mybir.dt
    F32, B16 = dt.float32, dt.bfloat16
    AF = mybir.ActivationFunctionType

    nc = bacc.Bacc("TRN2", target_bir_lowering=False, debug=False,
                   enable_asserts=False, num_devices=NCORES)

    # ---- DRAM I/O ----
    di = lambda n, sh, d: nc.dram_tensor(n, sh, d, kind="ExternalInput").ap()
    xt = di("xt", [ENC, BT], B16)         # X^T, cols t-major: [e, t*32+b]
    x = di("x", [BT, ENC], B16)           # X, rows b-major: [b*128+t, e]
    yb = di("yb", [4, S * BL], B16)       # rows [y0,y1,y2,1], cols s*32+b
    w2t = di("w2t", [ENC, ENC], B16)      # attn2_w.T [e, f]
    w1t = di("w1t", [2 * DEC, ENC], B16)  # attn1_w.T [k_hc, f]
    w3d = di("w3d", [128, 2048], B16)     # diag-masked w3 [f, fc*1024+b*32+(b&7)]
    bc = di("bc", [ENC, 1], F32)          # attn1_b + attn2_b
    wyb = di("wyb", [4, 4 * DEC], B16)    # [y0..2,1] -> gates (perm), row3=gbias
    wgc = di("wgc", [ENC, 4 * DEC], B16)  # ctx -> gates (perm)
    wgh = di("wgh", [DEC, 4 * DEC], B16)  # h -> gates (perm)
    fct = di("fct", [DEC + ENC, OUT], B16)  # (fc2@fc1).T
    fcb = di("fcb", [1, OUT], B16)
    onesr = di("onesr", [1, 512], B16)
    i128 = di("i128", [128, 128], F32)    # identity for transposes
    o = nc.dram_tensor("o", [OUT, S * BL], dt.float32, kind="ExternalOutput").ap()

    with tile.TileContext(nc) as tc, ExitStack() as ctx:
        # ---------------- persistent SBUF ----------------
        P = ctx.enter_context(tc.tile_pool(name="persist", bufs=1))
        Z2 = [P.tile([128, BT], B16, tag=f"z2{i}", name=f"Z2_{i}") for i in range(2)]
        TIN = [P.tile([128, BT], B16, tag=f"tin{i}", name=f"TIN_{i}") for i in range(2)]
        TOUT = [P.tile([128, BT], B16, tag=f"tout{i}", name=f"TOUT_{i}") for i in range(2)]
        XS = P.tile([128, BL * ENC], B16, tag="xs")          # [t, b*256+e]
        YBS = P.tile([4, S * BL], B16, tag="ybs")
        W1TS = P.tile([128, 4 * ENC], B16, tag="w1ts")       # [kc*256+fc*128+f]
        W3DS = P.tile([128, 2048], B16, tag="w3ds")
        BCS = P.tile([128, 2], F32, tag="bcs")
        WYBS = P.tile([4, 4 * DEC], B16, tag="wybs")
        WGCS = P.tile([128, 2 * 4 * DEC], B16, tag="wgcs")   # [ec*1024+g]
        WGHS = P.tile([128, 2 * 4 * DEC], B16, tag="wghs")
        FCTS = P.tile([128, 4 * OUT], B16, tag="fcts")
        FCBS = P.tile([1, OUT], B16, tag="fcbs")
        ONES = P.tile([1, 512], B16, tag="ones")
        I128 = P.tile([128, 128], F32, tag="i128")
        TH = [P.tile([128, S * BL], B16, tag=f"th{i}", name=f"TH_{i}") for i in range(4)]
        DIAG = P.tile([128, BL * 32], B16, tag="diag")       # ctx lhsT arena
        ZB16 = P.tile([128, 32], B16, tag="zb16")            # zero state step0
        CF32 = [P.tile([128, 64], F32, tag=f"cf{i}", name=f"CF_{i}") for i in range(2)]
        CB16 = [P.tile([128, 64], B16, tag=f"cb{i}", name=f"CB_{i}") for i in range(2)]

        # load weights / inputs
        for b in range(BL):
            nc.sync.dma_start(XS[:, b * ENC:(b + 1) * ENC], x[b * T:(b + 1) * T, :])
        nc.sync.dma_start(YBS[:], yb[:])
        for kc in range(4):
            nc.sync.dma_start(W1TS[:, kc * ENC:(kc + 1) * ENC],
                              w1t[kc * 128:(kc + 1) * 128, :])
        nc.sync.dma_start(W3DS[:], w3d[:])
        for c in range(2):
            nc.sync.dma_start(BCS[:, c:c + 1], bc[c * 128:(c + 1) * 128, :])
        nc.sync.dma_start(WYBS[:], wyb[:])
        for j in range(2):
            nc.sync.dma_start(WGCS[:, j * 1024:(j + 1) * 1024],
                              wgc[j * 128:(j + 1) * 128, :])
            nc.sync.dma_start(WGHS[:, j * 1024:(j + 1) * 1024],
                              wgh[j * 128:(j + 1) * 128, :])
        for kc in range(4):
            nc.sync.dma_start(FCTS[:, kc * OUT:(kc + 1) * OUT],
                              fct[kc * 128:(kc + 1) * 128, :])
        nc.sync.dma_start(FCBS[:], fcb[:])
        nc.sync.dma_start(ONES[:], onesr[:])
        nc.sync.dma_start(I128[:], i128[:])

        nc.vector.memset(DIAG[:], 0.0)
        nc.vector.memset(ZB16[:], 0.0)
        nc.vector.memset(CF32[0][:], 0.0)
        nc.vector.memset(CB16[0][:], 0.0)

        # ---------------- z2 precompute (bias folded in) ----------------
        with tc.tile_pool(name="xts", bufs=1) as xtp, \
             tc.tile_pool(name="z2ps", bufs=2, space="PSUM") as z2ps, \
             tc.tile_pool(name="w2p", bufs=1) as w2p:
            W2TS = w2p.tile([128, 2 * ENC], B16)
            for ec in range(2):
                nc.sync.dma_start(W2TS[:, ec * ENC:(ec + 1) * ENC],
                                  w2t[ec * 128:(ec + 1) * 128, :])
            XTS = [xtp.tile([128, BT], B16, tag=f"xt{e}", name=f"XTS_{e}") for e in range(2)]
            for ec in range(2):
                nc.sync.dma_start(XTS[ec][:], xt[ec * 128:(ec + 1) * 128, :])
            for fc in range(2):
                for nq in range(8):
                    zp = z2ps.tile([128, 512], F32, tag="zp")
                    for ec in range(2):
                        nc.tensor.matmul(
                            zp[:], W2TS[:, ec * ENC + fc * 128: ec * ENC + fc * 128 + 128],
                            XTS[ec][:, nq * 512:(nq + 1) * 512],
                            start=(ec == 0), stop=(ec == 1))
                    nc.scalar.activation(Z2[fc][:, nq * 512:(nq + 1) * 512], zp[:],
                                         AF.Identity, bias=BCS[:, fc:fc + 1])

        # ---------------- step pools ----------------
        loop_ctx = ExitStack()
        sb_p = loop_ctx.enter_context(tc.tile_pool(name="small", bufs=2))
        st_p = loop_ctx.enter_context(tc.tile_pool(name="state", bufs=2))
        z1_ps = loop_ctx.enter_context(tc.tile_pool(name="z1ps", bufs=1, space="PSUM"))
        sc_ps = loop_ctx.enter_context(tc.tile_pool(name="scps", bufs=2, space="PSUM"))
        et_ps = loop_ctx.enter_context(tc.tile_pool(name="etps", bufs=1, space="PSUM"))
        cx_ps = loop_ctx.enter_context(tc.tile_pool(name="cxps", bufs=1, space="PSUM"))
        ct_ps = loop_ctx.enter_context(tc.tile_pool(name="ctps", bufs=1, space="PSUM"))
        g_ps = loop_ctx.enter_context(tc.tile_pool(name="gps", bufs=2, space="PSUM"))

        for s in range(S):
            last = (s == S - 1)
            hprev = ([ZB16, ZB16] if s == 0 else
                     [TH[0][:, (s - 1) * 32: s * 32], TH[1][:, (s - 1) * 32: s * 32]])
            cprevb = ZB16 if s == 0 else None   # bf16 c for z1 rhs
            cb = CB16[s % 2]
            cf = CF32[s % 2]

            # ---- z1 = W1 @ hc, f-major [f, (fc,b)] ----
            z1p = z1_ps.tile([128, 64], F32, tag="z1")
            for fc in range(2):
                for kc in range(4):
                    if kc < 2:
                        rhs = hprev[kc]
                    else:
                        rhs = (ZB16[:] if s == 0
                               else cb[:, (kc - 2) * 32:(kc - 1) * 32])
                    nc.tensor.matmul(
                        z1p[:, fc * 32:(fc + 1) * 32],
                        W1TS[:, kc * ENC + fc * 128: kc * ENC + fc * 128 + 128],
                        rhs, start=(fc == 0 and kc == 0), stop=(fc == 1 and kc == 3))
            z1s = sb_p.tile([128, 64], B16, tag="z1s")
            nc.vector.tensor_copy(z1s[:], z1p[:])

            # ---- gates early parts: y+bias, h (PE busy during tanh) ----
            if not last:
                gp = g_ps.tile([128, 256], F32, tag="g")
                for j in range(8):
                    nc.tensor.matmul(gp[:, j * 32:(j + 1) * 32],
                                     WYBS[:, j * 128:(j + 1) * 128],
                                     YBS[:, s * 32:(s + 1) * 32],
                                     start=(j == 0), stop=False)
                for half in range(2):
                    for j in range(8):
                        nc.tensor.matmul(gp[:, j * 32:(j + 1) * 32],
                                         WGHS[:, half * 1024 + j * 128:
                                              half * 1024 + (j + 1) * 128],
                                         hprev[half],
                                         start=False, stop=False)

            # ---- tanh(z1 + z2): DVE bcast add + ACT (2 t-chunks/half) ----
            for fc in range(2):
                tin3 = TIN[fc][:].rearrange("p (t b) -> p t b", b=32)
                z23 = Z2[fc][:].rearrange("p (t b) -> p t b", b=32)
                z1b = z1s[:, None, fc * 32:(fc + 1) * 32].broadcast_to([128, T, 32])
                nc.vector.tensor_add(tin3, z23, z1b)
            for fc in range(2):
                for tc2 in range(2):
                    nc.scalar.activation(
                        TOUT[fc][:, tc2 * 2048:(tc2 + 1) * 2048],
                        TIN[fc][:, tc2 * 2048:(tc2 + 1) * 2048], AF.Tanh)

            # ---- scores: diag-lhsT MMs, 4-way column-packed ----
            # psum [128,128]: row 32g+i = scores for b=8g+i
            scp = sc_ps.tile([128, 128], F32, tag="sc")
            for tc2 in range(2):
                for fc in range(2):
                    t3 = TOUT[fc][:].rearrange("p (t b) -> p t b", b=32)
                    for i in range(8):
                        for g in range(4):
                            b = 8 * g + i
                            nc.tensor.matmul(
                                scp[32 * g:32 * (g + 1), tc2 * 64:(tc2 + 1) * 64],
                                W3DS[:, fc * 1024 + b * 32: fc * 1024 + b * 32 + 32],
                                t3[:, tc2 * 64:(tc2 + 1) * 64, b],
                                start=(tc2 == 0 and fc == 0 and i == 0),
                                stop=(tc2 == 1 and fc == 1 and i == 7),
                                tile_position=(0, 32 * g),
                                skip_group_check=True)

            # ---- softmax pieces ----
            E = sb_p.tile([128, 128], F32, tag="E")
            D = sb_p.tile([128, 1], F32, tag="D")
            nc.scalar.activation(E[:], scp[:], AF.Exp, accum_out=D[:])
            Dinv = sb_p.tile([128, 1], F32, tag="Dinv")
            nc.vector.reciprocal(Dinv[:], D[:])

            # E^T via PE transpose, then strided diag write (col b*32+(b&7))
            etp = et_ps.tile([128, 128], F32, tag="et")
            nc.tensor.transpose(etp[:], E[:], I128[:])
            diag3 = DIAG[:].rearrange("p (g c) -> p g c", c=256)
            et3 = etp[:].rearrange("p (g c) -> p g c", c=32)
            nc.vector.tensor_copy(diag3[:, :, 0:232:33], et3[:, :, 0:8])

            # ---- context: 4-way column-packed accumulating MMs ----
            cxp = cx_ps.tile([128, ENC], F32, tag="cx")
            for i in range(8):
                for g in range(4):
                    b = 8 * g + i
                    nc.tensor.matmul(
                        cxp[32 * g:32 * (g + 1), :],
                        DIAG[:, b * 32:(b + 1) * 32],
                        XS[:, b * ENC:(b + 1) * ENC],
                        start=(i == 0), stop=(i == 7),
                        tile_position=(0, 32 * g),
                        skip_group_check=True)
            # Dinv-scaled copy to SBUF (per-partition scalar on DVE)
            cxs = sb_p.tile([128, ENC], F32, tag="cxs")
            nc.vector.tensor_scalar_mul(cxs[:], cxp[:], Dinv[:])

            # ctx^T -> TH slots (strided col extract, cast bf16)
            ctp = ct_ps.tile([128, 256], F32, tag="ctp")
            for half in range(2):
                nc.tensor.transpose(ctp[:, half * 128:(half + 1) * 128],
                                    cxs[:, half * 128:(half + 1) * 128], I128[:])
            for half in range(2):
                src = ctp[:, half * 128:(half + 1) * 128].rearrange(
                    "p (g c) -> p g c", c=32)[:, :, 0:8]
                dst = TH[2 + half][:, s * 32:(s + 1) * 32].rearrange(
                    "p (g i) -> p g i", i=8)
                nc.vector.tensor_copy(dst, src)

            if last:
                # h2_127 == h_126: copy previous th h-slots
                for j in range(2):
                    nc.vector.tensor_copy(TH[j][:, s * 32:(s + 1) * 32],
                                          TH[j][:, (s - 1) * 32: s * 32])
                break

            # ---- gates ctx part ----
            for half in range(2):
                for j in range(8):
                    nc.tensor.matmul(gp[:, j * 32:(j + 1) * 32],
                                     WGCS[:, half * 1024 + j * 128:
                                          half * 1024 + (j + 1) * 128],
                                     TH[2 + half][:, s * 32:(s + 1) * 32],
                                     start=False, stop=(half == 1 and j == 7))

            # ---- LSTM elementwise, f-major [128, 64] = (dchunk, b) ----
            # sigmoid via tanh: sig(x) = (1+tanh(x/2))/2 -- keeps ACT on the
            # exp_and_others table set (no per-step ACT_TABLE_LOAD).
            # States stored doubled: c' = 2c, h' = 2h; consumers of h/c
            # (W1TS, WGHS, fct h-rows, tanh-c scale) pre-scaled by 0.5.
            # gp col layout: i 0:64, f 64:128, o 128:192, g 192:256
            OP = mybir.AluOpType
            sif = st_p.tile([128, 192], F32, tag="sif")
            nc.scalar.activation(sif[:], gp[:, 0:192], AF.Tanh, scale=0.5)
            tg = st_p.tile([128, 64], F32, tag="tg")
            nc.scalar.activation(tg[:], gp[:, 192:256], AF.Tanh)
            # a = (tau_f+1)*c'; b = (tau_i+1)*g~; c'_new = 0.5*a + b
            t1 = st_p.tile([128, 64], F32, tag="t1")
            nc.vector.scalar_tensor_tensor(t1[:], sif[:, 64:128], 1.0, cf[:],
                                           OP.add, OP.mult)
            t2 = st_p.tile([128, 64], F32, tag="t2")
            nc.vector.scalar_tensor_tensor(t2[:], sif[:, 0:64], 1.0, tg[:],
                                           OP.add, OP.mult)
            cn = CF32[(s + 1) % 2]
            nc.vector.scalar_tensor_tensor(cn[:], t1[:], 0.5, t2[:],
                                           OP.mult, OP.add)
            tc_ = st_p.tile([128, 64], F32, tag="tc")
            nc.scalar.activation(tc_[:], cn[:], AF.Tanh, scale=0.5)
            # h' = (tau_o+1)*tanh(c)
            hn = st_p.tile([128, 64], F32, tag="hn")
            nc.vector.scalar_tensor_tensor(hn[:], sif[:, 128:192], 1.0, tc_[:],
                                           OP.add, OP.mult)

            # state casts: h -> TH slots (bf16), c -> CB16
            for j in range(2):
                nc.vector.tensor_copy(TH[j][:, s * 32:(s + 1) * 32],
                                      hn[:, j * 32:(j + 1) * 32])
            nc.vector.tensor_copy(CB16[(s + 1) % 2][:], cn[:])

        loop_ctx.close()

        # ---------------- output head ----------------
        with tc.tile_pool(name="ops", bufs=2, space="PSUM") as ops, \
             tc.tile_pool(name="ost", bufs=2) as ost:
            for nq in range(8):
                op = ops.tile([OUT, 512], F32, tag="op")
                for kc in range(4):
                    nc.tensor.matmul(op[:], FCTS[:, kc * OUT:(kc + 1) * OUT],
                                     TH[kc][:, nq * 512:(nq + 1) * 512],
                                     start=(kc == 0), stop=False)
                nc.tensor.matmul(op[:], FCBS[:], ONES[:],
                                 start=False, stop=True)
                ot = ost.tile([OUT, 512], F32, tag="ot")
                nc.vector.tensor_copy(ot[:], op[:])
                nc.sync.dma_start(o[:, nq * 512:(nq + 1) * 512], ot[:])

    nc.compile()
    return nc


def _host_prep(inputs):
    f32 = np.float32
    ie = np.asarray(inputs["input_encoded"], f32)      # [256,128,256]
    ys = np.asarray(inputs["y_seq"], f32)              # [256,128,3]
    a1w = np.asarray(inputs["attn1_w"], f32)           # [256,512]
    a1b = np.asarray(inputs["attn1_b"], f32)
    a2w = np.asarray(inputs["attn2_w"], f32)
    a2b = np.asarray(inputs["attn2_b"], f32)
    a3w = np.asarray(inputs["attn3_w"], f32)           # [1,256]
    tw = np.asarray(inputs["tilde_w"], f32)            # [512,259]
    tb = np.asarray(inputs["tilde_b"], f32)
    wih = np.asarray(inputs["w_ih"], f32)              # [1024,512]
    whh = np.asarray(inputs["w_hh"], f32)              # [1024,256]
    bih = np.asarray(inputs["b_ih"], f32)
    bhh = np.asarray(inputs["b_hh"], f32)
    f1w = np.asarray(inputs["fc1_w"], f32)             # [256,512]
    f1b = np.asarray(inputs["fc1_b"], f32)
    f2w = np.asarray(inputs["fc2_w"], f32)             # [3,256]
    f2b = np.asarray(inputs["fc2_b"], f32)

    wcomb = wih @ tw                                    # [1024,259]
    gbias = wih @ tb + bih + bhh                        # [1024]
    fc = f2w @ f1w                                      # [3,512]
    fcbias = f2w @ f1b + f2b                            # [3]

    # gate permutation: psum col-blocks ordered (i, f, o, g)
    perm = np.concatenate([np.arange(0, 512),           # i, f
                           np.arange(768, 1024),        # o
                           np.arange(512, 768)])        # g
    wcombT = np.ascontiguousarray(wcomb.T)[:, perm]     # [259,1024]
    whhT = np.ascontiguousarray(whh.T)[:, perm]         # [256,1024]
    gbias_p = gbias[perm]

    wyb = np.concatenate([wcombT[0:3], gbias_p[None, :]], axis=0)  # [4,1024]

    # w3 diag arena: column (b & 7) within each b's 32-col slice
    w3diag = np.zeros((128, 2, 32, 32), f32)
    for fc_ in range(2):
        for b in range(32):
            w3diag[:, fc_, b, b & 7] = a3w[0, fc_ * 128:(fc_ + 1) * 128]
    w3diag = w3diag.reshape(128, 2048)

    # h/c states are stored doubled on device (tau-form LSTM): pre-scale
    # every consumer of h'/c' by 0.5.
    fcT = np.ascontiguousarray(fc.T).copy()
    fcT[0:256] *= 0.5
    shared = {
        "w2t": np.ascontiguousarray(a2w.T).astype(BF16),
        "w1t": (np.ascontiguousarray(a1w.T) * 0.5).astype(BF16),
        "w3d": w3diag.astype(BF16),
        "bc": (a1b + a2b)[:, None].astype(f32),
        "wyb": wyb.astype(BF16),
        "wgc": np.ascontiguousarray(wcombT[3:259]).astype(BF16),
        "wgh": (whhT * 0.5).astype(BF16),
        "fct": fcT.astype(BF16),
        "fcb": fcbias[None, :].astype(BF16),
        "onesr": np.ones((1, 512), BF16),
        "i128": np.eye(128, dtype=f32),
    }
    in_maps = []
    for i in range(NCORES):
        b0 = i * BL
        xe = ie[b0:b0 + BL]                            # [32,128,256]
        m = dict(shared)
        m["xt"] = np.ascontiguousarray(
            xe.transpose(2, 1, 0).reshape(ENC, BT)).astype(BF16)
        m["x"] = xe.reshape(BT, ENC).astype(BF16)
        yt = ys[b0:b0 + BL].transpose(2, 1, 0).reshape(OUT, S * BL)
        m["yb"] = np.concatenate(
            [yt, np.ones((1, S * BL), f32)], axis=0).astype(BF16)
        in_maps.append(m)
    return in_maps


def kernel(**inputs):
    global _BUILT
    from concourse import bass_utils
    if _BUILT is None:
        _BUILT = _build_nc()
    nc = _BUILT
    import os
    in_maps = _host_prep(inputs)
    trace = bool(int(os.environ.get("KERNEL_TRACE", "0")))
    res = bass_utils.run_bass_kernel_spmd(nc, in_maps, core_ids=list(range(NCORES)),
                                          trace=trace)
    if trace:
        print(f"HW exec time: {res.exec_time_ns} ns  (mean {res.mean_exec_time_ns})")
        globals()['_LAST_RESULTS'] = res
    outs = []
    for i in range(NCORES):
        oc = res.results[i]["o"]                       # [3, 4096] (j, s*32+b)
        outs.append(oc.reshape(OUT, S, BL).transpose(2, 1, 0))
    return np.concatenate(outs, axis=0).astype(np.float32)


if __name__ == "__main__":
    pass


# revision 22
# speedup vs baseline: 2.5087x; 1.1558x over previous
"""Trainium2 Bass kernel for nn_Decoder (Bahdanau-attention LSTM decoder).

B=256,T=128,ENC=DEC=256,OUT=3. Data-parallel over batch: 8 cores x 32 batch.

v4 design (per core): two independent 16-batch groups software-pipelined
half a step apart, so one group's ScalarE tanh overlaps the other group's
back-half (softmax/ctx/gates/LSTM).  Emission order per step:
  BACK(g1, s-1), FRONT(g0, s), BACK(g0, s), FRONT(g1, s)
FRONT = z1, gates y/bias/h parts, bcast-add, tanh, scores, exp.
BACK  = E^T, diag, ctx, Dinv-scale, ctx^T, gates ctx part, LSTM.

Attention matmuls are 4-way column-packed (tile_position col-groups, 4
batches each).  Gates and LSTM are computed transposed (feature-major), so
no per-step state transposes are needed.  Sigmoid is computed via
tanh(x/2) identities (states stored doubled, consumers pre-scaled 0.5) so
ScalarE never leaves the exp/tanh table set.  PSUM co-tenancy packs each
group's step state into 3 banks (collision-safety via dependency chains).
"""

import sys
import numpy as np

sys.path.insert(0, "/opt/trn_rl_repo")

import ml_dtypes

BF16 = ml_dtypes.bfloat16

NCORES = 8
BL = 32          # batch per core
GB = 16          # batch per pipeline group
T = 128          # encoder positions == decoder steps
ENC = 256
DEC = 256
OUT = 3
BT = BL * T      # 4096
GT = GB * T      # 2048
S = 128          # decoder steps

_BUILT = None


def _build_nc():
    from contextlib import ExitStack
    from concourse import bacc, mybir, tile

    dt = mybir.dt
    F32, B16 = dt.float32, dt.bfloat16
    AF = mybir.ActivationFunctionType
    OP = mybir.AluOpType

    nc = bacc.Bacc("TRN2", target_bir_lowering=False, debug=False,
                   enable_asserts=False, num_devices=NCORES)

    di = lambda n, sh, d: nc.dram_tensor(n, sh, d, kind="ExternalInput").ap()
    xt = di("xt", [ENC, BT], B16)         # X^T, cols (g, t, b')
    x = di("x", [BT, ENC], B16)           # X, rows b*128+t
    yb = di("yb", [4, S * BL], B16)       # rows [y0,y1,y2,1], cols s*32+b
    w2t = di("w2t", [ENC, ENC], B16)
    w1t = di("w1t", [2 * DEC, ENC], B16)
    w3d = di("w3d", [128, 2048], B16)     # [f, fc*1024+b*32+((b%16)&3)]
    bc = di("bc", [ENC, 1], F32)
    wyb = di("wyb", [4, 4 * DEC], B16)
    wgc = di("wgc", [ENC, 4 * DEC], B16)
    wgh = di("wgh", [DEC, 4 * DEC], B16)
    fct = di("fct", [DEC + ENC, OUT], B16)
    fcb = di("fcb", [1, OUT], B16)
    onesr = di("onesr", [1, 512], B16)
    i128 = di("i128", [128, 128], F32)
    selm = di("selm", [128, 16], B16)     # col b'=4a+i selects row 32a+i
    o = nc.dram_tensor("o", [OUT, S * BL], dt.float32, kind="ExternalOutput").ap()

    with tile.TileContext(nc) as tc, ExitStack() as ctx:
        # ---------------- persistent SBUF ----------------
        P = ctx.enter_context(tc.tile_pool(name="persist", bufs=1))
        Z2 = [P.tile([128, BT], B16, tag=f"z2{i}", name=f"Z2_{i}") for i in range(2)]
        TIN = [P.tile([128, BT], B16, tag=f"tin{i}", name=f"TIN_{i}") for i in range(2)]
        TOUT = [P.tile([128, BT], B16, tag=f"tout{i}", name=f"TOUT_{i}") for i in range(2)]
        XS = P.tile([128, BL * ENC], B16, tag="xs")
        YBS = P.tile([4, S * BL], B16, tag="ybs")
        W1TS = P.tile([128, 4 * ENC], B16, tag="w1ts")
        W3DS = P.tile([128, 2048], B16, tag="w3ds")
        BCS = P.tile([128, 2], F32, tag="bcs")
        WYBS = P.tile([4, 4 * DEC], B16, tag="wybs")
        WGCS = P.tile([128, 2 * 4 * DEC], B16, tag="wgcs")
        WGHS = P.tile([128, 2 * 4 * DEC], B16, tag="wghs")
        FCTS = P.tile([128, 4 * OUT], B16, tag="fcts")
        FCBS = P.tile([1, OUT], B16, tag="fcbs")
        ONES = P.tile([1, 512], B16, tag="ones")
        I128 = P.tile([128, 128], F32, tag="i128")
        SELB = P.tile([128, 16], B16, tag="selb")
        TH = [P.tile([128, S * BL], B16, tag=f"th{i}", name=f"TH_{i}") for i in range(4)]
        DIAG = P.tile([128, BL * 32], B16, tag="diag")
        ZB16 = P.tile([128, 16], B16, tag="zb16")
        CF32 = [[P.tile([128, 32], F32, tag=f"cf{g}{i}", name=f"CF_{g}_{i}")
                 for i in range(2)] for g in range(2)]
        CB16 = [[P.tile([128, 32], B16, tag=f"cb{g}{i}", name=f"CB_{g}_{i}")
                 for i in range(2)] for g in range(2)]

        for b in range(BL):
            nc.sync.dma_start(XS[:, b * ENC:(b + 1) * ENC], x[b * T:(b + 1) * T, :])
        nc.sync.dma_start(YBS[:], yb[:])
        for kc in range(4):
            nc.sync.dma_start(W1TS[:, kc * ENC:(kc + 1) * ENC],
                              w1t[kc * 128:(kc + 1) * 128, :])
        nc.sync.dma_start(W3DS[:], w3d[:])
        for c in range(2):
            nc.sync.dma_start(BCS[:, c:c + 1], bc[c * 128:(c + 1) * 128, :])
        nc.sync.dma_start(WYBS[:], wyb[:])
        for j in range(2):
            nc.sync.dma_start(WGCS[:, j * 1024:(j + 1) * 1024],
                              wgc[j * 128:(j + 1) * 128, :])
            nc.sync.dma_start(WGHS[:, j * 1024:(j + 1) * 1024],
                              wgh[j * 128:(j + 1) * 128, :])
        for kc in range(4):
            nc.sync.dma_start(FCTS[:, kc * OUT:(kc + 1) * OUT],
                              fct[kc * 128:(kc + 1) * 128, :])
        nc.sync.dma_start(FCBS[:], fcb[:])
        nc.sync.dma_start(ONES[:], onesr[:])
        nc.sync.dma_start(I128[:], i128[:])
        nc.sync.dma_start(SELB[:], selm[:])

        nc.vector.memset(DIAG[:], 0.0)
        nc.vector.memset(ZB16[:], 0.0)
        for g in range(2):
            nc.vector.memset(CF32[g][0][:], 0.0)
            nc.vector.memset(CB16[g][0][:], 0.0)

        # ---------------- z2 precompute (bias folded in) ----------------
        with tc.tile_pool(name="xts", bufs=1) as xtp, \
             tc.tile_pool(name="z2ps", bufs=2, space="PSUM") as z2ps, \
             tc.tile_pool(name="w2p", bufs=1) as w2p:
            W2TS = w2p.tile([128, 2 * ENC], B16)
            for ec in range(2):
                nc.sync.dma_start(W2TS[:, ec * ENC:(ec + 1) * ENC],
                                  w2t[ec * 128:(ec + 1) * 128, :])
            XTS = [xtp.tile([128, BT], B16, tag=f"xt{e}", name=f"XTS_{e}")
                   for e in range(2)]
            for ec in range(2):
                nc.sync.dma_start(XTS[ec][:], xt[ec * 128:(ec + 1) * 128, :])
            for fc in range(2):
                for nq in range(8):
                    zp = z2ps.tile([128, 512], F32, tag="zp")
                    for ec in range(2):
                        nc.tensor.matmul(
                            zp[:], W2TS[:, ec * ENC + fc * 128: ec * ENC + fc * 128 + 128],
                            XTS[ec][:, nq * 512:(nq + 1) * 512],
                            start=(ec == 0), stop=(ec == 1))
                    nc.scalar.activation(Z2[fc][:, nq * 512:(nq + 1) * 512], zp[:],
                                         AF.Identity, bias=BCS[:, fc:fc + 1])

        # ---------------- step pools ----------------
        loop_ctx = ExitStack()
        sb_p = loop_ctx.enter_context(tc.tile_pool(name="small", bufs=2))
        st_p = loop_ctx.enter_context(tc.tile_pool(name="state", bufs=2))
        # per-group psum co-tenant tiles, bufs=1 (3 banks per group)
        scz_p = [loop_ctx.enter_context(tc.tile_pool(name=f"scz{g}", bufs=1, space="PSUM"))
                 for g in range(2)]
        tp_p = [loop_ctx.enter_context(tc.tile_pool(name=f"tp{g}", bufs=1, space="PSUM"))
                for g in range(2)]
        cx_p = [loop_ctx.enter_context(tc.tile_pool(name=f"cx{g}", bufs=1, space="PSUM"))
                for g in range(2)]
        gp_p = [loop_ctx.enter_context(tc.tile_pool(name=f"gp{g}", bufs=1, space="PSUM"))
                for g in range(2)]
        SCZ = [scz_p[g].tile([128, 160], F32, tag=f"scz{g}", name=f"SCZ_{g}")
               for g in range(2)]
        TP = [tp_p[g].tile([128, 48], F32, tag=f"tp{g}", name=f"TP_{g}")
              for g in range(2)]
        CX = [cx_p[g].tile([128, 256], F32, tag=f"cx{g}", name=f"CX_{g}")
              for g in range(2)]
        GP = [gp_p[g].tile([128, 128], F32, tag=f"gp{g}", name=f"GP_{g}")
              for g in range(2)]

        mm = nc.tensor.matmul
        # per-group live sbuf tiles produced by FRONT, consumed by BACK
        live = [{}, {}]

        def fa(g, s):
            c0 = s * BL + g * GB
            hprev = ([ZB16[:], ZB16[:]] if s == 0 else
                     [TH[0][:, c0 - BL: c0 - BL + GB], TH[1][:, c0 - BL: c0 - BL + GB]])
            live[g]["hprev"] = hprev
            cb = CB16[g][s % 2]

            # z1 = W1 @ hc  [128, (fc,b')]  -> SCZ cols 0:32
            for fc in range(2):
                for kc in range(4):
                    rhs = (hprev[kc] if kc < 2
                           else (ZB16[:] if s == 0
                                 else cb[:, (kc - 2) * 16:(kc - 1) * 16]))
                    mm(SCZ[g][:, fc * 16:(fc + 1) * 16],
                       W1TS[:, kc * ENC + fc * 128: kc * ENC + fc * 128 + 128],
                       rhs, start=(fc == 0 and kc == 0), stop=(fc == 1 and kc == 3),
                       skip_group_check=True)
            z1s = sb_p.tile([128, 32], B16, tag=f"z1s{g}")
            nc.vector.tensor_copy(z1s[:], SCZ[g][:, 0:32])

            # bcast add + tanh on the group's 2048-col slice
            gsl = slice(g * GT, (g + 1) * GT)
            for fc in range(2):
                tin3 = TIN[fc][:, gsl].rearrange("p (t b) -> p t b", b=GB)
                z23 = Z2[fc][:, gsl].rearrange("p (t b) -> p t b", b=GB)
                z1b = z1s[:, None, fc * 16:(fc + 1) * 16].broadcast_to([128, T, GB])
                nc.vector.tensor_add(tin3, z23, z1b)
            for fc in range(2):
                nc.scalar.activation(TOUT[fc][:, gsl], TIN[fc][:, gsl], AF.Tanh)

        def fb(g, s):
            last = (s == S - 1)
            c0 = s * BL + g * GB
            hprev = live[g]["hprev"]
            gsl = slice(g * GT, (g + 1) * GT)

            # gates early parts (y+bias, h)
            if not last:
                for j in range(8):
                    mm(GP[g][:, j * 16:(j + 1) * 16],
                       WYBS[:, j * 128:(j + 1) * 128],
                       YBS[:, c0:c0 + GB],
                       start=(j == 0), stop=False, skip_group_check=True)
                for half in range(2):
                    for j in range(8):
                        mm(GP[g][:, j * 16:(j + 1) * 16],
                           WGHS[:, half * 1024 + j * 128: half * 1024 + (j + 1) * 128],
                           hprev[half], start=False, stop=False,
                           skip_group_check=True)

            # scores: diag-lhsT, 4-way column-packed; psum row 32*cg+i = b'=4cg+i
            for tc2 in range(2):
                for fc in range(2):
                    t3 = TOUT[fc][:, gsl].rearrange("p (t b) -> p t b", b=GB)
                    for i in range(4):
                        for cg in range(4):
                            bp = 4 * cg + i
                            b = g * GB + bp
                            mm(SCZ[g][32 * cg:32 * (cg + 1),
                                      32 + tc2 * 64: 96 + tc2 * 64],
                               W3DS[:, fc * 1024 + b * 32: fc * 1024 + b * 32 + 32],
                               t3[:, tc2 * 64:(tc2 + 1) * 64, bp],
                               start=(tc2 == 0 and fc == 0 and i == 0),
                               stop=(tc2 == 1 and fc == 1 and i == 3),
                               tile_position=(0, 32 * cg), skip_group_check=True)

            E = sb_p.tile([128, 128], B16, tag=f"E{g}")
            D = sb_p.tile([128, 1], F32, tag=f"D{g}")
            nc.scalar.activation(E[:], SCZ[g][:, 32:160], AF.Exp, accum_out=D[:])
            Dinv = sb_p.tile([128, 1], F32, tag=f"Di{g}")
            nc.vector.reciprocal(Dinv[:], D[:])
            live[g]["E"] = E
            live[g]["Dinv"] = Dinv

        def back(g, s):
            last = (s == S - 1)
            c0 = s * BL + g * GB
            E, Dinv = live[g]["E"], live[g]["Dinv"]

            # E^T selected columns via tiny matmul: out[t, b'] = E[slot(b'), t]
            mm(TP[g][:, 0:16], E[:], SELB[:], start=True, stop=True,
               skip_group_check=True)
            dg3 = DIAG[:, g * 512:(g + 1) * 512].rearrange("p (a c) -> p a c", c=128)
            et3 = TP[g][:, 0:16].rearrange("p (a c) -> p a c", c=4)
            nc.vector.tensor_copy(dg3[:, :, 0:100:33], et3[:])

            for i in range(4):
                for cg in range(4):
                    b = g * GB + 4 * cg + i
                    mm(CX[g][32 * cg:32 * (cg + 1), :],
                       DIAG[:, b * 32:(b + 1) * 32],
                       XS[:, b * ENC:(b + 1) * ENC],
                       start=(i == 0), stop=(i == 3),
                       tile_position=(0, 32 * cg), skip_group_check=True)
            cxs = sb_p.tile([128, ENC], B16, tag=f"cxs{g}")
            nc.vector.tensor_scalar_mul(cxs[:], CX[g][:], Dinv[:])

            for half in range(2):
                mm(TP[g][:, 16 + half * 16: 32 + half * 16],
                   cxs[:, half * 128:(half + 1) * 128], SELB[:],
                   start=True, stop=True, skip_group_check=True)
            for half in range(2):
                nc.vector.tensor_copy(TH[2 + half][:, c0:c0 + GB],
                                      TP[g][:, 16 + half * 16: 32 + half * 16])

            if last:
                for j in range(2):
                    nc.vector.tensor_copy(TH[j][:, c0:c0 + GB],
                                          TH[j][:, c0 - BL: c0 - BL + GB])
                return

            for half in range(2):
                for j in range(8):
                    mm(GP[g][:, j * 16:(j + 1) * 16],
                       WGCS[:, half * 1024 + j * 128: half * 1024 + (j + 1) * 128],
                       TH[2 + half][:, c0:c0 + GB],
                       start=False, stop=(half == 1 and j == 7),
                       skip_group_check=True)

            # LSTM elementwise (tau-form), [128, 32] = (dchunk, b')
            cf = CF32[g][s % 2]
            # one ACT call: tau for i,f,o and tanh(g) (g-weights doubled on host)
            sifg = st_p.tile([128, 128], F32, tag=f"sif{g}")
            nc.scalar.activation(sifg[:], GP[g][:], AF.Tanh, scale=0.5)
            sif = sifg[:, 0:96]
            tg = sifg[:, 96:128]
            t1 = st_p.tile([128, 32], F32, tag=f"t1{g}")
            nc.vector.scalar_tensor_tensor(t1[:], sifg[:, 32:64], 1.0, cf[:],
                                           OP.add, OP.mult)
            t2 = st_p.tile([128, 32], F32, tag=f"t2{g}")
            nc.vector.scalar_tensor_tensor(t2[:], sifg[:, 0:32], 1.0, tg,
                                           OP.add, OP.mult)
            cn = CF32[g][(s + 1) % 2]
            nc.vector.scalar_tensor_tensor(cn[:], t1[:], 0.5, t2[:],
                                           OP.mult, OP.add)
            tc_ = st_p.tile([128, 32], F32, tag=f"tc{g}")
            nc.scalar.activation(tc_[:], cn[:], AF.Tanh, scale=0.5)
            hn = st_p.tile([128, 32], F32, tag=f"hn{g}")
            nc.vector.scalar_tensor_tensor(hn[:], sifg[:, 64:96], 1.0, tc_[:],
                                           OP.add, OP.mult)
            for j in range(2):
                nc.vector.tensor_copy(TH[j][:, c0:c0 + GB],
                                      hn[:, j * 16:(j + 1) * 16])
            nc.vector.tensor_copy(CB16[g][(s + 1) % 2][:], cn[:])

        # staggered pipeline; adds/tanh (fa) emitted first to feed ScalarE
        for s in range(S):
            fa(0, s)
            if s > 0:
                back(1, s - 1)
            fb(0, s)
            fa(1, s)
            back(0, s)
            fb(1, s)
        back(1, S - 1)

        loop_ctx.close()

        # ---------------- output head ----------------
        with tc.tile_pool(name="ops", bufs=2, space="PSUM") as ops, \
             tc.tile_pool(name="ost", bufs=2) as ost:
            for nq in range(8):
                op = ops.tile([OUT, 512], F32, tag="op")
                for kc in range(4):
                    mm(op[:], FCTS[:, kc * OUT:(kc + 1) * OUT],
                       TH[kc][:, nq * 512:(nq + 1) * 512],
                       start=(kc == 0), stop=False)
                mm(op[:], FCBS[:], ONES[:], start=False, stop=True)
                ot = ost.tile([OUT, 512], F32, tag="ot")
                nc.vector.tensor_copy(ot[:], op[:])
                nc.sync.dma_start(o[:, nq * 512:(nq + 1) * 512], ot[:])

    nc.compile()
    return nc


def _host_prep(inputs):
    f32 = np.float32
    ie = np.asarray(inputs["input_encoded"], f32)
    ys = np.asarray(inputs["y_seq"], f32)
    a1w = np.asarray(inputs["attn1_w"], f32)
    a1b = np.asarray(inputs["attn1_b"], f32)
    a2w = np.asarray(inputs["attn2_w"], f32)
    a2b = np.asarray(inputs["attn2_b"], f32)
    a3w = np.asarray(inputs["attn3_w"], f32)
    tw = np.asarray(inputs["tilde_w"], f32)
    tb = np.asarray(inputs["tilde_b"], f32)
    wih = np.asarray(inputs["w_ih"], f32)
    whh = np.asarray(inputs["w_hh"], f32)
    bih = np.asarray(inputs["b_ih"], f32)
    bhh = np.asarray(inputs["b_hh"], f32)
    f1w = np.asarray(inputs["fc1_w"], f32)
    f1b = np.asarray(inputs["fc1_b"], f32)
    f2w = np.asarray(inputs["fc2_w"], f32)
    f2b = np.asarray(inputs["fc2_b"], f32)

    wcomb = wih @ tw
    gbias = wih @ tb + bih + bhh
    fc = f2w @ f1w
    fcbias = f2w @ f1b + f2b

    perm = np.concatenate([np.arange(0, 512),
                           np.arange(768, 1024),
                           np.arange(512, 768)])
    wcombT = np.ascontiguousarray(wcomb.T)[:, perm]
    whhT = np.ascontiguousarray(whh.T)[:, perm]
    gbias_p = gbias[perm]
    wyb = np.concatenate([wcombT[0:3], gbias_p[None, :]], axis=0)

    # w3 diag arena: column ((b%16)&3) within each b's 32-col slice
    w3diag = np.zeros((128, 2, 32, 32), f32)
    for fc_ in range(2):
        for b in range(32):
            w3diag[:, fc_, b, (b % 16) & 3] = a3w[0, fc_ * 128:(fc_ + 1) * 128]
    w3diag = w3diag.reshape(128, 2048)

    # double the g-gate columns so one tanh(x/2) ACT call serves i,f,o,g
    wcombT[:, 768:1024] *= 2.0
    whhT[:, 768:1024] *= 2.0
    gbias_p = gbias_p.copy()
    gbias_p[768:1024] *= 2.0
    wyb = np.concatenate([wcombT[0:3], gbias_p[None, :]], axis=0)

    selm = np.zeros((128, 16), np.float32)
    for bp in range(16):
        selm[32 * (bp >> 2) + (bp & 3), bp] = 1.0

    fcT = np.ascontiguousarray(fc.T).copy()
    fcT[0:256] *= 0.5
    shared = {
        "w2t": np.ascontiguousarray(a2w.T).astype(BF16),
        "w1t": (np.ascontiguousarray(a1w.T) * 0.5).astype(BF16),
        "w3d": w3diag.astype(BF16),
        "bc": (a1b + a2b)[:, None].astype(f32),
        "wyb": wyb.astype(BF16),
        "wgc": np.ascontiguousarray(wcombT[3:259]).astype(BF16),
        "wgh": (whhT * 0.5).astype(BF16),
        "fct": fcT.astype(BF16),
        "fcb": fcbias[None, :].astype(BF16),
        "onesr": np.ones((1, 512), BF16),
        "i128": np.eye(128, dtype=f32),
        "selm": selm.astype(BF16),
    }
    in_maps = []
    for i in range(NCORES):
        b0 = i * BL
        xe = ie[b0:b0 + BL]                            # [32,128,256]
        m = dict(shared)
        # xt cols ordered (g, t, b'): group-contiguous 2048-col halves
        m["xt"] = np.ascontiguousarray(
            xe.reshape(2, GB, T, ENC).transpose(3, 0, 2, 1).reshape(ENC, BT)
        ).astype(BF16)
        m["x"] = xe.reshape(BT, ENC).astype(BF16)
        yt = ys[b0:b0 + BL].transpose(2, 1, 0).reshape(OUT, S * BL)
        m["yb"] = np.concatenate(
            [yt, np.ones((1, S * BL), f32)], axis=0).astype(BF16)
        in_maps.append(m)
    return in_maps


def kernel(**inputs):
    global _BUILT
    from concourse import bass_utils
    if _BUILT is None:
        _BUILT = _build_nc()
    nc = _BUILT
    import os
    in_maps = _host_prep(inputs)
    trace = bool(int(os.environ.get("KERNEL_TRACE", "0")))
    res = bass_utils.run_bass_kernel_spmd(nc, in_maps, core_ids=list(range(NCORES)),
                                          trace=trace)
    if trace:
        print(f"HW exec time: {res.exec_time_ns} ns  (mean {res.mean_exec_time_ns})")
        globals()['_LAST_RESULTS'] = res
    outs = []
    for i in range(NCORES):
        oc = res.results[i]["o"]                       # [3, 4096] (j, s*32+b)
        outs.append(oc.reshape(OUT, S, BL).transpose(2, 1, 0))
    return np.concatenate(outs, axis=0).astype(np.float32)


if __name__ == "__main__":
    pass
